# revision 68
# baseline (speedup 1.0000x reference)
"""Trainium2 Bass kernel for BilinearInteraction.

out[b, p] = x[b, i_p, :] @ W[p] @ x[b, j_p, :]  for the 780 field pairs
(i, j), i < j, of F=40 fields (row-major triu order).

Architecture (8 NeuronCores, data-parallel over batch, B_loc=256).
Hybrid two-layout design balancing all four compute engines:

  NT path (fields i >= IT, pair-e on partitions / batch on free dim):
    stage 1: PE matmul Y[(2 pair, e), b] = Wtile.T @ xT_i (bf16, K=64).
    stage 2 (per 4-tile / 2-bank PSUM group, mode-cycled):
      - AD: ACT evicts PSUM -> bf16, DVE multiplies by the xtc chunk at
        the 2x packed rate.
      - D:  DVE multiplies PSUM f32 directly (1x) -> bf16 z.
      - AP: ACT evicts, Pool (gpsimd) multiplies.
    stage 3: PE ones-mask matmuls accumulate 16 z-tiles into one PSUM
      bank = 32 output pair rows; ACT/DVE evict; DMA out.

  T path (fields i < IT, batch on partitions / (pair, e) on free dim):
    stage 1: PE matmul YT[b128, (pair, e)] = xtlo_i.T @ W_p (stationary
      is x, moving is W; per batch half; pairs ordered by (j, i) so the
      stage-2 multiplier is one broadcast x[:, j] column per j-run).
      Only ONE PE pass per pair -- no reduce matmuls on this path.
    stage 2 (per 8-pair / 1-bank PSUM tile, mode-cycled): multiply by
      x[b, j, e]: AP (ACT evict + Pool mult) / AD / DD as above.
    stage 3: log-tree fold over e on the DVE at the 2x bf16 rate
      (64->4 per 48-pair batch, 4->1 per half at the end), f32 result.

  Scheduling: the two paths are interleaved 1:1 so the per-step engine
  mix stays flat; the critical DMA quarters are issued before the bulk;
  and every cross-engine consumer (stage-2 multiplies behind ACT
  evicts, fold chains behind Pool multiplies, output-bank evictions
  behind the final reduce matmul) is EMITTED with a one-to-two unit lag
  so it never enters its engine's in-order queue with an unmet
  dependency (head-of-line blocking was worth ~15us). Engine busy under
  the TimelineSim cost model: ACT 85us / DVE 84 / PE 74 / Pool 70,
  wall 102.9us vs 119.4us for the single-layout baseline (HW-verified,
  rel err 7e-3). Steady-state occupancy is ACT 97-100% / DVE 85-100%;
  the residual over the engine floor is the DMA-bound fill window, the
  PE p-state ramp, and the drain tail.
"""

import numpy as np
import ml_dtypes

import concourse.bass as bass
import concourse.mybir as mybir
import concourse.tile as tile
from concourse import bacc
from concourse.bass_utils import run_bass_kernel_spmd

B, F, D = 2048, 40, 64
P = F * (F - 1) // 2  # 780
NCORES = 8
B_LOC = B // NCORES  # 256
HB = B_LOC // 2  # 128 (batch half on T-path partitions)
F32 = mybir.dt.float32
BF16 = mybir.dt.bfloat16

NCHUNK = F // 2  # 20 xT chunks (2 fields each)

# ---- tunables -------------------------------------------------------------
IT = 7  # fields 0..IT-1 go to the T path
NT_MODE_CYCLE = ["AD", "D", "AD", "D", "AD", "D", "AP", "AD", "AP"]
T_MODE_CYCLE = ["AP", "AD", "AP", "DD"]
REDUCE_DELAY = 11  # NT reduce-matmul lag, in groups (software pipelining)
GMAX = 4  # NT tiles per PSUM group (2 banks)
TILES_PER_BANK = 16
BANK_ROWS = 32
TP_TILE = 8  # T pairs per PSUM tile (1 bank)
TBATCH = 48  # T pairs per fold batch
FOLD_POOL_EVERY = 0  # every Nth fold batch runs its first fold level on Pool
MDEFER = 2  # deferral depth for evict-dependent multiplies
WARMUP_MM = 0  # dep-free dummy matmuls to ramp the PE p-state at t=0
WDMA_NT = 16  # NT stage-1 lhsT tiles per DMA (128 cols each)
WDMA_T = 32  # T pairs per W DMA (64 cols each)
# ---------------------------------------------------------------------------


def configure(**kw):
    """Set tunables and recompute all derived schedule tables."""
    g = globals()
    for k, v in kw.items():
        assert k in g, k
        g[k] = v

    # T pairs: (i, j) for i < IT, grouped by j (i ascending). Within a
    # j-run the stage-2 multiplier is ONE broadcast x[:, j] column; early
    # tiles touch only low fields (short DMA critical path).
    g["T_PAIRS"] = [(i, j) for j in range(1, F) for i in range(min(j, IT))]
    g["PT"] = len(T_PAIRS)

    # NT tiles: (t, i) covering pairs (i, 2t) [dummy if i==2t] and (i, 2t+1)
    g["NT_TILES"] = [(t, i) for t in range(NCHUNK) for i in range(IT, 2 * t + 1)]
    g["NTILES"] = len(NT_TILES)
    g["NBANKS"] = (NTILES + TILES_PER_BANK - 1) // TILES_PER_BANK
    g["OUT_ROWS_NT"] = NBANKS * BANK_ROWS

    g["WT_COLS"] = PT * D  # T region of W (pair-major [d, (p, e)])
    g["WNT_COLS"] = NTILES * 128  # NT region (tile-major [d, (2 pair, e)])

    # NT groups: GMAX same-chunk tiles sharing one 2-bank PSUM group;
    # split at reduce-bank boundaries
    groups = []
    k = 0
    for t in range(NCHUNK):
        ilist = list(range(IT, 2 * t + 1))
        while ilist:
            take = min(GMAX, len(ilist), TILES_PER_BANK - (k % TILES_PER_BANK))
            groups.append((t, ilist[:take]))
            ilist = ilist[take:]
            k += take
    g["NT_GROUPS"] = groups

    # T tiles: consecutive TP_TILE-pair tiles with j-runs
    tiles = []
    s = 0
    while s < PT:
        n = min(TP_TILE, PT - s)
        runs = []  # (j, pair offset within tile, count)
        k = s
        while k < s + n:
            j = T_PAIRS[k][1]
            run = 1
            while k + run < s + n and T_PAIRS[k + run][1] == j:
                run += 1
            runs.append((j, k - s, run))
            k += run
        tiles.append((s, n, runs))
        s += n
    g["T_TILES"] = tiles


T_PAIRS = PT = NT_TILES = NTILES = NBANKS = OUT_ROWS_NT = None
WT_COLS = WNT_COLS = NT_GROUPS = T_TILES = None
configure()


def host_prep(W: np.ndarray):
    """Build Wt [64, WT_COLS + WNT_COLS] bf16, ONES [128, 512] bf16,
    and output-row maps for both paths."""
    Wt2 = np.ascontiguousarray(W.transpose(1, 0, 2))  # [64, 780, 64]
    pair_idx = -np.ones((F, F), dtype=np.int64)
    k = 0
    for i in range(F):
        for j in range(i + 1, F):
            pair_idx[i, j] = k
            k += 1

    Wt = np.zeros((D, WT_COLS + WNT_COLS), dtype=np.float32)
    # T region: pair-major blocks
    t_pids = np.empty(PT, dtype=np.int64)
    for idx, (i, j) in enumerate(T_PAIRS):
        p = pair_idx[i, j]
        t_pids[idx] = p
        Wt[:, idx * D : (idx + 1) * D] = Wt2[:, p, :]
    # NT region: tile-major, zero blocks for dummy (i == 2t) slots
    nt_rows = []
    nt_pids = []
    for kk, (t, i) in enumerate(NT_TILES):
        jA, jB = 2 * t, 2 * t + 1
        base = WT_COLS + kk * 128
        bank, s = divmod(kk, TILES_PER_BANK)
        rowA = bank * BANK_ROWS + 2 * s
        if i < jA:
            Wt[:, base : base + D] = Wt2[:, pair_idx[i, jA], :]
            nt_rows.append(rowA)
            nt_pids.append(pair_idx[i, jA])
        Wt[:, base + D : base + 128] = Wt2[:, pair_idx[i, jB], :]
        nt_rows.append(rowA + 1)
        nt_pids.append(pair_idx[i, jB])

    ones = np.zeros((128, 512), dtype=np.float32)
    for q in range(16):
        ones[0:64, q * 32 + 2 * q] = 1.0
        ones[64:128, q * 32 + 2 * q + 1] = 1.0
    return (
        Wt.astype(ml_dtypes.bfloat16),
        ones.astype(ml_dtypes.bfloat16),
        np.asarray(nt_rows),
        np.asarray(nt_pids),
        t_pids,
    )


def build_nc():
    nc = bacc.Bacc("TRN2", target_bir_lowering=False, debug=False)

    xtc_dram = nc.dram_tensor(
        "xtc", [128, NCHUNK * B_LOC], BF16, kind="ExternalInput"
    ).ap()
    xtlo_dram = nc.dram_tensor(
        "xtlo", [64, F * B_LOC], BF16, kind="ExternalInput"
    ).ap()
    # xnat[p, f*128 + h*64 + e] = x[h*128 + p, f, e] (T stage-2 multiplier)
    xnat_dram = nc.dram_tensor(
        "xnat", [128, 2 * F * D], BF16, kind="ExternalInput"
    ).ap()
    wt_dram = nc.dram_tensor(
        "Wt", [D, WT_COLS + WNT_COLS], BF16, kind="ExternalInput"
    ).ap()
    ones_dram = nc.dram_tensor("ones", [128, 512], BF16, kind="ExternalInput").ap()
    outnt_dram = nc.dram_tensor(
        "outNT", [OUT_ROWS_NT, B_LOC], F32, kind="ExternalOutput"
    ).ap()
    outt_dram = nc.dram_tensor("outT", [128, 2 * PT], F32, kind="ExternalOutput").ap()

    with tile.TileContext(nc) as tc:
        with (
            tc.tile_pool(name="persist", bufs=1) as persist,
            tc.tile_pool(name="wnt", bufs=6) as wntpool,
            tc.tile_pool(name="wt", bufs=6) as wtpool,
            tc.tile_pool(name="zpool", bufs=16) as zpool,
            tc.tile_pool(name="ybf", bufs=6) as ybfpool,
            tc.tile_pool(name="ybfT", bufs=6) as ybfTpool,
            tc.tile_pool(name="ztb", bufs=4) as ztbpool,
            tc.tile_pool(name="fold", bufs=4) as foldpool,
            tc.tile_pool(name="opool", bufs=4) as opool,
            tc.tile_pool(name="ynt", bufs=2, space=bass.MemorySpace.PSUM) as ynt,
            tc.tile_pool(name="yt", bufs=2, space=bass.MemorySpace.PSUM) as ytp,
            tc.tile_pool(name="rpsum", bufs=2, space=bass.MemorySpace.PSUM) as rpsum,
        ):
            ones = persist.tile([128, 512], BF16, tag="ones")
            xtc = persist.tile([128, NCHUNK * B_LOC], BF16, tag="xtc")
            xtlo = persist.tile([64, F * B_LOC], BF16, tag="xtlo")
            xnat = persist.tile([128, 2 * F * D], BF16, tag="xnat")
            nq = NCHUNK * B_LOC // 4
            nf = F * B_LOC // 4
            nn = 2 * F * D // 4

            def dma_quarter(dst, src, n1, c4):
                nc.sync.dma_start(
                    out=dst[:, c4 * n1 : (c4 + 1) * n1],
                    in_=src[:, c4 * n1 : (c4 + 1) * n1],
                )

            # PE p-state warmup: ~26 dependency-free dummy matmuls on a
            # never-written SBUF scratch keep the PE continuously busy from
            # t~0 so the 3us ramp to full clock completes before the first
            # real matmul's inputs land (the ramp resets on any idle gap).
            if WARMUP_MM:
                warm_sb = persist.tile([64, 512], BF16, tag="warm_sb")
                nc.gpsimd.memset(warm_sb[:], 0.0)
                warm_ps = ynt.tile(
                    [128, GMAX * B_LOC], F32, tag="y", name="warm"
                )
                for _ in range(WARMUP_MM):
                    nc.tensor.matmul(
                        warm_ps[:, 0:256],
                        warm_sb[:, 0:128],
                        warm_sb[:, 256:512],
                        start=True,
                        stop=True,
                    )

            # Critical-path-first DMA order: the first compute groups need
            # xtlo q0, the leading W chunks (emitted by the schedule), then
            # xtc q0 / xnat q0+q1 for the first multiplies. Bulk x follows.
            dma_quarter(xtlo, xtlo_dram, nf, 0)

            # z4[h]: per-half fold-4 accumulation (width 4 per pair)
            z4 = [
                persist.tile([128, PT * 4], BF16, tag=f"z4_{h}", name=f"z4_{h}")
                for h in range(2)
            ]
            outt_sb = persist.tile([128, 2 * PT], F32, tag="outt_sb")

            # ---- NT machinery --------------------------------------------
            rbs = [None]
            nt_pending = []
            ob_pending = []

            def flush_ob(all_=False):
                while len(ob_pending) > (0 if all_ else 1):
                    bank, rb = ob_pending.pop(0)
                    ob = opool.tile([BANK_ROWS, B_LOC], F32, tag="ob")
                    nc.scalar.copy(out=ob[:], in_=rb[0:BANK_ROWS, :])
                    nc.sync.dma_start(
                        out=outnt_dram[
                            bank * BANK_ROWS : (bank + 1) * BANK_ROWS, :
                        ],
                        in_=ob[:],
                    )
            eng_acc = {"ACT": 0.0, "DVE": 0.0, "POOL": 0.0}
            NT_COST = {
                "AD": {"ACT": 1223, "DVE": 594},
                "D": {"DVE": 1192},
                "AP": {"ACT": 1223, "POOL": 2068},
            }
            T_COST = {
                "AP": {"ACT": 797, "POOL": 1112, "DVE": 300},
                "AD": {"ACT": 797, "DVE": 687},
                "DD": {"DVE": 958},
            }

            def pick_mode(costs, allowed):
                best, bestm = None, None
                for m in allowed:
                    mx = max(
                        eng_acc[e] + costs[m].get(e, 0) for e in eng_acc
                    )
                    if best is None or mx < best:
                        best, bestm = mx, m
                return bestm

            def charge(costs, m):
                for e, v in costs[m].items():
                    eng_acc[e] += v

            def reduce_unit(z, idx, kt):
                bank, s = divmod(kt, TILES_PER_BANK)
                if s == 0:
                    rbs[0] = rpsum.tile([128, B_LOC], F32, tag="rb", name="rb")
                rb = rbs[0]
                last = (s == TILES_PER_BANK - 1) or (kt == NTILES - 1)
                nc.tensor.matmul(
                    rb[0:32, :],
                    ones[:, s * 32 : (s + 1) * 32],
                    z[:, idx * B_LOC : (idx + 1) * B_LOC],
                    start=(s == 0),
                    stop=last,
                    tile_position=(0, 0),
                    skip_group_check=True,
                )
                if last:
                    ob_pending.append((bank, rb))
                    flush_ob()

            def maybe_reduce_unit(force=False):
                # interleave single reduce matmuls between stage-1 matmuls
                # so no 4 consecutive PE instructions share an unmet dep
                if nt_pending and (force or len(nt_pending) > 4 * REDUCE_DELAY):
                    reduce_unit(*nt_pending.pop(0))

            wnt_state = {"chunks": [], "next": 0}  # [(base, size, tile)]
            nt_k = [0]
            nt_gi = [0]
            nt_mult_q = []
            t_mult_q = []

            def ensure_wnt(kt):
                while kt >= wnt_state["next"]:
                    k0 = wnt_state["next"]
                    nw = min(WDMA_NT, NTILES - k0)
                    wchunk = wntpool.tile([64, WDMA_NT * 128], BF16, tag="w")
                    nc.sync.dma_start(
                        out=wchunk[:, : nw * 128],
                        in_=wt_dram[
                            :, WT_COLS + k0 * 128 : WT_COLS + (k0 + nw) * 128
                        ],
                    )
                    wnt_state["chunks"] = wnt_state["chunks"][-2:] + [
                        (k0, nw, wchunk)
                    ]
                    wnt_state["next"] = k0 + nw

            def wnt_lookup(kt):
                for base, size, tile_ in wnt_state["chunks"]:
                    if base <= kt < base + size:
                        return base, tile_
                raise AssertionError(f"wnt chunk for tile {kt} evicted")

            def emit_nt_group(t, ilist):
                gsz = len(ilist)
                k0 = nt_k[0]
                if NT_MODE_CYCLE == ["greedy"]:
                    mode = pick_mode(NT_COST, ["AD", "D", "AP"])
                    charge(NT_COST, mode)
                else:
                    mode = NT_MODE_CYCLE[nt_gi[0] % len(NT_MODE_CYCLE)]
                nt_gi[0] += 1
                y = ynt.tile([128, GMAX * B_LOC], F32, tag="y")
                for idx, i in enumerate(ilist):
                    kt = k0 + idx
                    ensure_wnt(kt)
                    cb, wtile_nt = wnt_lookup(kt)
                    kk = kt - cb
                    nc.tensor.matmul(
                        y[:, idx * B_LOC : (idx + 1) * B_LOC],
                        wtile_nt[:, kk * 128 : (kk + 1) * 128],
                        xtlo[:, i * B_LOC : (i + 1) * B_LOC],
                        start=True,
                        stop=True,
                    )

                z = zpool.tile([128, GMAX * B_LOC], BF16, tag="z")
                xin = xtc[:, None, t * B_LOC : (t + 1) * B_LOC].to_broadcast(
                    [128, gsz, B_LOC]
                )
                zr = z[:, : gsz * B_LOC].rearrange("p (n b) -> p n b", n=gsz)
                if mode in ("AD", "AP"):
                    ybf = ybfpool.tile([128, GMAX * B_LOC], BF16, tag="ybf")
                    nc.scalar.copy(
                        out=ybf[:, : gsz * B_LOC], in_=y[:, : gsz * B_LOC]
                    )
                    yr = ybf[:, : gsz * B_LOC].rearrange("p (n b) -> p n b", n=gsz)
                    eng = nc.vector if mode == "AD" else nc.gpsimd

                    def mult_thunk(eng=eng, zr=zr, yr=yr, xin=xin):
                        eng.tensor_tensor(zr, yr, xin, mybir.AluOpType.mult)

                    nt_mult_q.append(mult_thunk)
                    while len(nt_mult_q) > MDEFER:
                        nt_mult_q.pop(0)()
                else:  # D: DVE direct from PSUM f32 (frees y -> emit now)
                    yr = y[:, : gsz * B_LOC].rearrange("p (n b) -> p n b", n=gsz)
                    nc.vector.tensor_tensor(zr, yr, xin, mybir.AluOpType.mult)

                for idx in range(gsz):
                    nt_pending.append((z, idx, k0 + idx))
                nt_k[0] += gsz
                for _ in range(gsz):
                    maybe_reduce_unit()

            # ---- T machinery ---------------------------------------------
            wt_state = {"chunks": [], "next": 0}  # [(base, size, tile)]
            t_ti = [0]
            # fold batches: per half, (ztile, filled-pairs) or None
            batch_open = [None, None]
            batch_start = [0, 0]

            fold_queue = [[], []]
            fold_ctr = [0]

            def flush_batch(h, drain=False):
                if batch_open[h] is not None:
                    fold_queue[h].append((batch_open[h], batch_start[h]))
                    batch_start[h] = batch_start[h] + batch_open[h][1]
                    batch_open[h] = None
                while len(fold_queue[h]) > (0 if drain else 1):
                    (zt, n), bstart = fold_queue[h].pop(0)
                    emit_folds(h, zt, n, bstart)

            def emit_folds(h, zt, n, bstart):
                fold_ctr[0] += 1
                f1eng = (
                    nc.gpsimd
                    if FOLD_POOL_EVERY and fold_ctr[0] % FOLD_POOL_EVERY == 0
                    else nc.vector
                )
                f1 = foldpool.tile([128, TBATCH * 32], BF16, tag="f1")
                f1eng.tensor_tensor(
                    f1[:, : n * 32].rearrange("p (n w) -> p n w", n=n),
                    zt[:, : n * 64].rearrange("p (n w) -> p n w", n=n)[:, :, 0:32],
                    zt[:, : n * 64].rearrange("p (n w) -> p n w", n=n)[:, :, 32:64],
                    mybir.AluOpType.add,
                )
                f2 = foldpool.tile([128, TBATCH * 16], BF16, tag="f2")
                nc.vector.tensor_tensor(
                    f2[:, : n * 16].rearrange("p (n w) -> p n w", n=n),
                    f1[:, : n * 32].rearrange("p (n w) -> p n w", n=n)[:, :, 0:16],
                    f1[:, : n * 32].rearrange("p (n w) -> p n w", n=n)[:, :, 16:32],
                    mybir.AluOpType.add,
                )
                f3 = foldpool.tile([128, TBATCH * 8], BF16, tag="f3")
                nc.vector.tensor_tensor(
                    f3[:, : n * 8].rearrange("p (n w) -> p n w", n=n),
                    f2[:, : n * 16].rearrange("p (n w) -> p n w", n=n)[:, :, 0:8],
                    f2[:, : n * 16].rearrange("p (n w) -> p n w", n=n)[:, :, 8:16],
                    mybir.AluOpType.add,
                )
                nc.vector.tensor_tensor(
                    z4[h][:, bstart * 4 : (bstart + n) * 4].rearrange(
                        "p (n w) -> p n w", n=n
                    ),
                    f3[:, : n * 8].rearrange("p (n w) -> p n w", n=n)[:, :, 0:4],
                    f3[:, : n * 8].rearrange("p (n w) -> p n w", n=n)[:, :, 4:8],
                    mybir.AluOpType.add,
                )

            def ensure_wt(s):
                while s >= wt_state["next"]:
                    nxt = wt_state["next"]
                    nwp = min(WDMA_T, PT - nxt)
                    wchunk = wtpool.tile([64, WDMA_T * D], BF16, tag="wt")
                    nc.sync.dma_start(
                        out=wchunk[:, : nwp * D],
                        in_=wt_dram[:, nxt * D : (nxt + nwp) * D],
                    )
                    wt_state["chunks"] = wt_state["chunks"][-2:] + [
                        (nxt, nwp, wchunk)
                    ]
                    wt_state["next"] = nxt + nwp

            def wt_lookup(s):
                for base, size, tile_ in wt_state["chunks"]:
                    if base <= s < base + size:
                        return base, tile_
                raise AssertionError(f"wt chunk for pair {s} evicted")

            def emit_t_tile(s, n, runs, h):
                if T_MODE_CYCLE == ["greedy"]:
                    mode = pick_mode(T_COST, ["AP", "AD", "DD"])
                    charge(T_COST, mode)
                else:
                    mode = T_MODE_CYCLE[t_ti[0] % len(T_MODE_CYCLE)]
                t_ti[0] += 1
                ensure_wt(s + n - 1)
                cbase, wtile = wt_lookup(s)
                wbase = (s - cbase) * D
                y = ytp.tile([128, TP_TILE * D], F32, tag="yt")
                for idx in range(n):
                    i = T_PAIRS[s + idx][0]
                    nc.tensor.matmul(
                        y[:, idx * D : (idx + 1) * D],
                        xtlo[:, i * B_LOC + h * HB : i * B_LOC + (h + 1) * HB],
                        wtile[:, wbase + idx * D : wbase + (idx + 1) * D],
                        start=True,
                        stop=True,
                    )

                # batch tile: tiles accumulate into a shared ztb tile
                if batch_open[h] is None:
                    batch_open[h] = (
                        ztbpool.tile([128, TBATCH * D], BF16, tag="ztb", name="ztb"),
                        0,
                    )
                zt, filled = batch_open[h]
                zoff = filled

                if mode in ("AD", "AP"):
                    ybf = ybfTpool.tile([128, TP_TILE * D], BF16, tag="ybfT")
                    nc.scalar.copy(out=ybf[:, : n * D], in_=y[:, : n * D])
                    src = ybf
                    eng = nc.vector if mode == "AD" else nc.gpsimd
                    defer = True
                else:
                    src = y
                    eng = nc.vector
                    defer = False

                def mults(eng=eng, src=src, zt=zt, zoff=zoff, runs=runs, h=h):
                    for j, off, cnt in runs:
                        xj = xnat[
                            :, None, (j * 2 + h) * D : (j * 2 + h + 1) * D
                        ].to_broadcast([128, cnt, D])
                        eng.tensor_tensor(
                            zt[
                                :, (zoff + off) * D : (zoff + off + cnt) * D
                            ].rearrange("p (n e) -> p n e", n=cnt),
                            src[:, off * D : (off + cnt) * D].rearrange(
                                "p (n e) -> p n e", n=cnt
                            ),
                            xj,
                            mybir.AluOpType.mult,
                        )

                if defer:
                    t_mult_q.append(mults)
                    while len(t_mult_q) > MDEFER:
                        t_mult_q.pop(0)()
                else:
                    mults()
                batch_open[h] = (zt, filled + n)
                if filled + n >= TBATCH or s + n >= PT:
                    flush_batch(h)
                for _ in range(2):
                    maybe_reduce_unit()

            # ---- merged schedule -----------------------------------------
            ntg = list(NT_GROUPS)
            ttl = []
            for s, n, runs in T_TILES:
                for h in range(2):
                    ttl.append((s, n, runs, h))
            # Critical path first: the leading W chunks and the x quarters
            # the first few groups touch, then the bulk spread over the
            # early schedule steps (all comfortably before first use).
            ensure_wnt(0)
            ensure_wt(0)
            dma_quarter(xtc, xtc_dram, nq, 0)
            dma_quarter(xtlo, xtlo_dram, nf, 1)
            dma_quarter(xtc, xtc_dram, nq, 1)
            dma_quarter(xnat, xnat_dram, nn, 0)
            deferred = [
                lambda: nc.sync.dma_start(out=ones[:], in_=ones_dram[:]),
                lambda: dma_quarter(xnat, xnat_dram, nn, 1),
                lambda: dma_quarter(xtlo, xtlo_dram, nf, 2),
                lambda: dma_quarter(xtc, xtc_dram, nq, 2),
                lambda: dma_quarter(xnat, xnat_dram, nn, 2),
                lambda: dma_quarter(xtlo, xtlo_dram, nf, 3),
                lambda: dma_quarter(xtc, xtc_dram, nq, 3),
                lambda: dma_quarter(xnat, xnat_dram, nn, 3),
            ]
            ni, ti = 0, 0
            nsteps = max(len(ntg), len(ttl))
            for step in range(nsteps):
                if ni < len(ntg) and ni <= step * len(ntg) // nsteps:
                    emit_nt_group(*ntg[ni])
                    ni += 1
                while ti < len(ttl) and ti <= step * len(ttl) // nsteps:
                    emit_t_tile(*ttl[ti])
                    ti += 1
                if deferred:
                    deferred.pop(0)()
            while ni < len(ntg):
                emit_nt_group(*ntg[ni]); ni += 1
            while ti < len(ttl):
                emit_t_tile(*ttl[ti]); ti += 1

            while nt_mult_q:
                nt_mult_q.pop(0)()
            while t_mult_q:
                t_mult_q.pop(0)()
            while nt_pending:
                maybe_reduce_unit(force=True)
            flush_ob(all_=True)
            for h in range(2):
                flush_batch(h, drain=True)

            # final folds 4 -> 1 (f32) per half, then DMA
            for h in (0, 1):
                f5 = foldpool.tile([128, PT * 2], BF16, tag="f5")
                nc.vector.tensor_tensor(
                    f5[:].rearrange("p (n w) -> p n w", n=PT),
                    z4[h][:].rearrange("p (n w) -> p n w", n=PT)[:, :, 0:2],
                    z4[h][:].rearrange("p (n w) -> p n w", n=PT)[:, :, 2:4],
                    mybir.AluOpType.add,
                )
                nc.vector.tensor_tensor(
                    outt_sb[:, h * PT : (h + 1) * PT].rearrange(
                        "p (n w) -> p n w", n=PT
                    ),
                    f5[:].rearrange("p (n w) -> p n w", n=PT)[:, :, 0:1],
                    f5[:].rearrange("p (n w) -> p n w", n=PT)[:, :, 1:2],
                    mybir.AluOpType.add,
                )
            nc.sync.dma_start(out=outt_dram[:], in_=outt_sb[:])

    nc.compile()
    return nc


_NC = None
_PREP = None


def kernel(x: np.ndarray, W: np.ndarray) -> np.ndarray:
    global _NC, _PREP
    x = np.ascontiguousarray(np.asarray(x, dtype=np.float32))
    W = np.ascontiguousarray(np.asarray(W, dtype=np.float32))
    assert x.shape == (B, F, D) and W.shape == (P, D, D)

    Wt, ones, nt_rows, nt_pids, t_pids = host_prep(W)

    if _NC is None:
        _NC = build_nc()

    in_maps = []
    for c in range(NCORES):
        xs = x[c * B_LOC : (c + 1) * B_LOC]  # [256, 40, 64]
        v = xs.transpose(1, 2, 0).reshape(NCHUNK, 2, D, B_LOC)
        xtc = np.ascontiguousarray(
            v.transpose(1, 2, 0, 3).reshape(128, NCHUNK * B_LOC)
        ).astype(ml_dtypes.bfloat16)
        xtlo = np.ascontiguousarray(
            xs.transpose(2, 1, 0).reshape(D, F * B_LOC)
        ).astype(ml_dtypes.bfloat16)
        # xnat[p, f*128 + h*64 + e] = x[h*128 + p, f, e]
        xnat = np.ascontiguousarray(
            xs.reshape(2, HB, F, D).transpose(1, 2, 0, 3).reshape(HB, 2 * F * D)
        ).astype(ml_dtypes.bfloat16)
        in_maps.append(
            {"xtc": xtc, "xtlo": xtlo, "xnat": xnat, "Wt": Wt, "ones": ones}
        )
    res = run_bass_kernel_spmd(_NC, in_maps, core_ids=list(range(NCORES)))
    out = np.empty((B, P), dtype=np.float32)
    for c in range(NCORES):
        outNT = res.results[c]["outNT"]  # [OUT_ROWS_NT, B_LOC]
        outT = res.results[c]["outT"]  # [128, 2*PT]
        bsl = slice(c * B_LOC, (c + 1) * B_LOC)
        out[bsl, :][:, nt_pids] = outNT[nt_rows, :].T
        for h in range(2):
            hsl = slice(c * B_LOC + h * HB, c * B_LOC + (h + 1) * HB)
            out[hsl, :][:, t_pids] = outT[:, h * PT : (h + 1) * PT]
    return out
